# revision 32
# baseline (speedup 1.0000x reference)
"""Trainium2 Bass kernel for a binarized BasicBlock (BinConv3x3 + scale + sync-BN + residual).

Reference computation (NCHW, N=64, C=256, H=W=28):
    out = BN_train(scale * conv3x3(sign(x), sign(w))) + x

Strategy: data-parallel over batch across 8 NeuronCores (8 images/core).
  - host: binarize weights to fp8 e4m3 DoubleRow lhsT tiles, fold gamma/scale/beta
  - device per core:
      sign(x) -> zero-padded fp8 tiles [128cin, 2cib, 30, 30] per image (ScalarE)
      conv3x3 = 9 shifted fp8 DoubleRow matmuls (K=256 per matmul) accumulated
      in PSUM; 2-4 PSUM tiles share each weight load so the PE streams gapless
      PSUM evacuation on ScalarE with row-accumulate -> per-channel sum(z);
      sum(z^2) via VectorE square+reduce
      2KB AllGather of the partial sums across the 8 cores + local reduce
      (exact sync-BN; sums of +-1 dot products are exact integers in fp32)
      per-channel A,B finalization; apply out = A*z + B + x on ScalarE/VectorE
"""

import os
import sys

sys.path.insert(0, "/opt/trn_rl_repo")

import numpy as np
import ml_dtypes

import concourse.mybir as mybir
import concourse.tile as tile
from concourse import bacc
from concourse.bass_utils import run_bass_kernel_spmd

AF = mybir.ActivationFunctionType
ALU = mybir.AluOpType

N_CORES = 8
N_PER_CORE = 8          # images per core
C = 256                 # channels
CB = 2                  # channel blocks of 128
P = 128                 # partitions
H = W = 28
HW = H * W              # 784
HP = WP = 30            # padded spatial
HALF = 14               # output rows per matmul group
NFREE = HALF * W        # 392 free elems per matmul
BN_EPS = 1e-5
N_TOTAL_ELEMS = 64 * HW  # BN normalizer: N*H*W over the full batch

_CACHED = None


def _build_nc():
    nc = bacc.Bacc("TRN2", target_bir_lowering=False, debug=False,
                   num_devices=N_CORES)

    x_dram = nc.dram_tensor("x", [N_PER_CORE, CB, P, HW], mybir.dt.float32,
                            kind="ExternalInput")
    wb_dram = nc.dram_tensor("wb", [P, CB * 9, CB, P], mybir.dt.float8e4,
                             kind="ExternalInput")
    pp_dram = nc.dram_tensor("pp", [P, CB, 3], mybir.dt.float32,
                             kind="ExternalInput")
    out_dram = nc.dram_tensor("out", [N_PER_CORE, CB, P, HW], mybir.dt.float32,
                              kind="ExternalOutput")

    with tile.TileContext(nc) as tc:
        with (
            tc.tile_pool(name="const", bufs=1) as cpool,
            tc.tile_pool(name="xin", bufs=1) as xpool,
            tc.tile_pool(name="spad", bufs=1) as spool,
            tc.tile_pool(name="z", bufs=1) as zpool,
            tc.tile_pool(name="sq", bufs=2) as sqpool,
            tc.tile_pool(name="small", bufs=1) as mpool,
            tc.tile_pool(name="psum", bufs=8, space="PSUM") as psum,
            tc.tile_pool(name="dram", bufs=1, space="DRAM") as dram,
        ):
            wt = cpool.tile([P, CB * 9, CB, P], mybir.dt.float8e4)
            nc.sync.dma_start(wt[:], wb_dram[:])

            # Force the Sign ACT LUT load at kernel start (otherwise walrus
            # schedules it right before the first real sign, gating the PE).
            dummy_sg = cpool.tile([P, 1], mybir.dt.float8e4)
            nc.scalar.activation(dummy_sg[:], nc.const_aps.tensor(0.0, (P, 1)),
                                 AF.Sign)


            # per-image padded-sign tiles [128, cib, 30, 30] (fp8 for DoubleRow)
            xts = []
            sts = []
            pp = None
            for n in range(N_PER_CORE):
                st = spool.tile([P, CB, HP, WP], mybir.dt.float8e4,
                                name=f"spad{n}", tag=f"spad{n}")
                sts.append(st)

            for n in range(N_PER_CORE):
                for cb in range(CB):
                    t = n * CB + cb
                    nc.vector.memset(sts[n][:, cb], 0.0)
                    xt = xpool.tile([P, HW], mybir.dt.float32,
                                    name=f"xin{t}", tag=f"xin{t}")
                    nc.sync.dma_start(xt[:], x_dram[n, cb])
                    xts.append(xt)
                    # sign(x) into the interior of the zero-padded tile
                    nc.scalar.activation(sts[n][:, cb, 1:29, 1:29], xt[:], AF.Sign)
                if n == 0:
                    pp = cpool.tile([P, CB, 3], mybir.dt.float32)
                    nc.sync.dma_start(pp[:], pp_dram[:])

            # conv output, raw (unscaled) integer-valued sums
            z = zpool.tile([P, CB, N_PER_CORE, HW], mybir.dt.float32)
            # per-chunk row-sum partials (one column per psum tile)
            s1c = mpool.tile([P, CB, 2 * N_PER_CORE], mybir.dt.float32)
            s2c = mpool.tile([P, CB, 2 * N_PER_CORE], mybir.dt.float32)

            # Conv via fp8 DoubleRow: each matmul contracts both cin-blocks
            # (K=256) at once; several PSUM tiles accumulate per weight load
            # so each lhsT is reused and PE streams. Last groups are smaller
            # so the final stats (which gate the AllGather) finish sooner.
            GROUPS = [[(0, 0), (0, 1)],
                      [(1, 0), (1, 1)],
                      [(2, 0), (2, 1), (3, 0), (3, 1)],
                      [(4, 0), (4, 1), (5, 0), (5, 1)],
                      [(6, 0), (6, 1)],
                      [(7, 0), (7, 1)]]
            for g, units in enumerate(GROUPS):
                for cob in range(CB):
                    pss = [psum.tile([P, NFREE], mybir.dt.float32,
                                     name=f"ps_{g}_{cob}_{j}", tag="ps")
                           for j in range(len(units))]
                    for dh in range(3):
                        for dw in range(3):
                            w_ap = wt[:, cob * 9 + dh * 3 + dw, :, :]
                            first = (dh == 0 and dw == 0)
                            last = (dh == 2 and dw == 2)
                            for j, (n, half) in enumerate(units):
                                h0 = half * HALF
                                nc.tensor.matmul(
                                    pss[j][:],
                                    w_ap,
                                    sts[n][:, :, h0 + dh:h0 + dh + HALF,
                                           dw:dw + W],
                                    start=first,
                                    stop=last,
                                    perf_mode=mybir.MatmulPerfMode.DoubleRow,
                                )
                    for j, (n, half) in enumerate(units):
                        h0 = half * HALF
                        idx = n * 2 + half
                        zsl = z[:, cob, n, h0 * W:(h0 + HALF) * W]
                        nc.scalar.activation(
                            zsl, pss[j][:],
                            AF.Copy, accum_out=s1c[:, cob, idx:idx + 1])
                        sq = sqpool.tile([P, NFREE], mybir.dt.float32, tag="sq")
                        nc.vector.tensor_mul(sq[:], zsl, zsl)
                        nc.vector.tensor_reduce(
                            s2c[:, cob, idx:idx + 1], sq[:],
                            axis=mybir.AxisListType.X, op=ALU.add)

            # local stats -> [128, 4] = [s1_b0, s1_b1, s2_b0, s2_b1]
            cc_sb = mpool.tile([P, 4], mybir.dt.float32)
            nc.vector.tensor_reduce(cc_sb[:, 0:2], s1c[:],
                                    axis=mybir.AxisListType.X, op=ALU.add)
            nc.vector.tensor_reduce(cc_sb[:, 2:4], s2c[:],
                                    axis=mybir.AxisListType.X, op=ALU.add)

            # exact sync-BN: AllGather the 2KB of partial sums (lower latency
            # than AllReduce), then reduce the 8 rank contributions locally.
            cc_in = dram.tile([P, 4], mybir.dt.float32)
            ag_out = dram.tile([N_CORES, P, 4], mybir.dt.float32,
                               addr_space="Shared")
            nc.sync.dma_start(cc_in[:], cc_sb[:])
            nc.gpsimd.collective_compute(
                "AllGather", ALU.bypass,
                replica_groups=[list(range(N_CORES))],
                ins=[cc_in[:]],
                outs=[ag_out[:]],
            )
            tot8 = mpool.tile([P, N_CORES, 4], mybir.dt.float32)
            nc.sync.dma_start(tot8[:], ag_out[:].rearrange("r p c -> p r c"))
            tot = mpool.tile([P, 4], mybir.dt.float32)
            nc.vector.tensor_reduce(tot[:], tot8[:].rearrange("p r c -> p c r"),
                                    axis=mybir.AxisListType.X, op=ALU.add)

            # per-channel finalization:
            #   mu_z = S1/M ; var_z = S2/M - mu_z^2 ; var_y = scale^2*var_z
            #   A = gamma*scale/sqrt(var_y+eps) ; B = beta - A*mu_z
            inv = 1.0 / N_TOTAL_ELEMS
            mu = mpool.tile([P, CB], mybir.dt.float32)
            ez2 = mpool.tile([P, CB], mybir.dt.float32)
            m2 = mpool.tile([P, CB], mybir.dt.float32)
            varz = mpool.tile([P, CB], mybir.dt.float32)
            vary = mpool.tile([P, CB], mybir.dt.float32)
            stdv = mpool.tile([P, CB], mybir.dt.float32)
            rstd = mpool.tile([P, CB], mybir.dt.float32)
            A = mpool.tile([P, CB], mybir.dt.float32)
            t0 = mpool.tile([P, CB], mybir.dt.float32)
            B = mpool.tile([P, CB], mybir.dt.float32)

            nc.vector.tensor_scalar_mul(mu[:], tot[:, 0:2], inv)
            nc.vector.tensor_scalar_mul(ez2[:], tot[:, 2:4], inv)
            nc.vector.tensor_mul(m2[:], mu[:], mu[:])
            nc.vector.tensor_sub(varz[:], ez2[:], m2[:])
            nc.vector.tensor_mul(vary[:], varz[:], pp[:, :, 0])
            nc.vector.tensor_scalar_add(vary[:], vary[:], BN_EPS)
            nc.scalar.activation(stdv[:], vary[:], AF.Sqrt)
            nc.vector.reciprocal(rstd[:], stdv[:])
            nc.vector.tensor_mul(A[:], rstd[:], pp[:, :, 1])
            nc.vector.tensor_mul(t0[:], A[:], mu[:])
            nc.vector.tensor_sub(B[:], pp[:, :, 2], t0[:])

            # apply: out = A*z + B + x, then DMA out (ACT scale-bias pass
            # pipelined with DVE residual add; GpSimd stays off SBUF ports —
            # its elementwise ops contend with DVE for the shared port pair)
            for t in range(N_PER_CORE * CB):
                n, cb = t // CB, t % CB
                zs = z[:, cb, n, :]
                nc.scalar.activation(zs, zs, AF.Identity,
                                     scale=A[:, cb:cb + 1],
                                     bias=B[:, cb:cb + 1])
                nc.vector.tensor_add(zs, zs, xts[t][:])
                nc.sync.dma_start(out_dram[n, cb], zs)

    nc.compile()
    return nc


def _prep_shared(w, scale, gamma, beta):
    w = np.asarray(w, dtype=np.float32)
    scale = np.asarray(scale, dtype=np.float32).reshape(C)
    gamma = np.asarray(gamma, dtype=np.float32).reshape(C)
    beta = np.asarray(beta, dtype=np.float32).reshape(C)

    # DoubleRow lhsT[k, idx=(cob,dh,dw), r, m] = sign(w)[cob*128+m, r*128+k, dh, dw]
    # stored [k][idx][r][m] (contiguous per partition k) as fp8 e4m3.
    wsign = np.sign(w).astype(ml_dtypes.float8_e4m3)
    arr = wsign.reshape(CB, P, CB, P, 3, 3).transpose(3, 0, 4, 5, 2, 1)
    wb = np.ascontiguousarray(arr.reshape(P, CB * 9, CB, P))

    pp = np.empty((P, CB, 3), dtype=np.float32)
    for cb in range(CB):
        ch = slice(cb * P, (cb + 1) * P)
        pp[:, cb, 0] = scale[ch] * scale[ch]
        pp[:, cb, 1] = gamma[ch] * scale[ch]
        pp[:, cb, 2] = beta[ch]
    return wb, pp


def kernel(x, w, scale, gamma, beta):
    global _CACHED
    if _CACHED is None:
        _CACHED = _build_nc()
    nc = _CACHED

    x = np.asarray(x, dtype=np.float32)
    wb, pp = _prep_shared(w, scale, gamma, beta)

    in_maps = []
    for i in range(N_CORES):
        xs = x[i * N_PER_CORE:(i + 1) * N_PER_CORE]
        xs = np.ascontiguousarray(xs.reshape(N_PER_CORE, CB, P, HW))
        in_maps.append({"x": xs, "wb": wb, "pp": pp})

    trace = bool(int(os.environ.get("KERNEL_TRACE", "0")))
    kw = {}
    tdir = os.environ.get("KERNEL_TRACE_DIR")
    if trace and tdir:
        global _NCALL
        _NCALL = globals().get("_NCALL", 0) + 1
        tdir = os.path.join(tdir, f"call{_NCALL}")
        os.makedirs(tdir, exist_ok=True)
        kw["tmpdir"] = tdir
    res = run_bass_kernel_spmd(nc, in_maps, core_ids=list(range(N_CORES)),
                               trace=trace, **kw)
    if trace:
        globals()["LAST_EXEC_NS"] = res.exec_time_ns
        globals()["LAST_RESULTS"] = res

    out = np.empty((64, C, H, W), dtype=np.float32)
    for i in range(N_CORES):
        o = res.results[i]["out"].reshape(N_PER_CORE, C, H, W)
        out[i * N_PER_CORE:(i + 1) * N_PER_CORE] = o
    return out



# revision 33
# speedup vs baseline: 1.6034x; 1.6034x over previous
"""Trainium2 Bass kernel for a binarized BasicBlock (BinConv3x3 + scale + sync-BN + residual).

Reference computation (NCHW, N=64, C=256, H=W=28):
    out = BN_train(scale * conv3x3(sign(x), sign(w))) + x

Strategy: data-parallel over batch across 8 NeuronCores (8 images/core).
  - host: binarize weights to fp8 e4m3 DoubleRow lhsT tiles, fold gamma/scale/beta
  - device per core:
      sign(x) -> zero-padded fp8 tiles [128cin, 2cib, 30, 30] per image (ScalarE)
      conv3x3 = 9 shifted fp8 DoubleRow matmuls (K=256 per matmul) accumulated
      in PSUM; 2-4 PSUM tiles share each weight load so the PE streams gapless
      PSUM evacuation on ScalarE with row-accumulate -> per-channel sum(z);
      sum(z^2) via VectorE square+reduce
      2KB AllGather of the partial sums across the 8 cores + local reduce
      (exact sync-BN; sums of +-1 dot products are exact integers in fp32)
      per-channel A,B finalization; apply out = A*z + B + x on ScalarE/VectorE
"""

import os
import sys

sys.path.insert(0, "/opt/trn_rl_repo")

import numpy as np
import ml_dtypes

import concourse.mybir as mybir
import concourse.tile as tile
from concourse import bacc
from concourse.bass_utils import run_bass_kernel_spmd

AF = mybir.ActivationFunctionType
ALU = mybir.AluOpType

N_CORES = 8
N_PER_CORE = 8          # images per core
C = 256                 # channels
CB = 2                  # channel blocks of 128
P = 128                 # partitions
H = W = 28
HW = H * W              # 784
HP = WP = 30            # padded spatial
HALF = 14               # output rows per matmul group
NFREE = HALF * W        # 392 free elems per matmul
BN_EPS = 1e-5
N_TOTAL_ELEMS = 64 * HW  # BN normalizer: N*H*W over the full batch

_CACHED = None


def _build_nc():
    nc = bacc.Bacc("TRN2", target_bir_lowering=False, debug=False,
                   num_devices=N_CORES)

    x_dram = nc.dram_tensor("x", [N_PER_CORE, CB, P, HW], mybir.dt.float32,
                            kind="ExternalInput")
    wb_dram = nc.dram_tensor("wb", [P, CB * 9, CB, P], mybir.dt.float8e4,
                             kind="ExternalInput")
    pp_dram = nc.dram_tensor("pp", [P, CB, 3], mybir.dt.float32,
                             kind="ExternalInput")
    out_dram = nc.dram_tensor("out", [N_PER_CORE, CB, P, HW], mybir.dt.float32,
                              kind="ExternalOutput")

    with tile.TileContext(nc) as tc:
        with (
            tc.tile_pool(name="const", bufs=1) as cpool,
            tc.tile_pool(name="xin", bufs=1) as xpool,
            tc.tile_pool(name="spad", bufs=1) as spool,
            tc.tile_pool(name="z", bufs=1) as zpool,
            tc.tile_pool(name="sq", bufs=2) as sqpool,
            tc.tile_pool(name="small", bufs=1) as mpool,
            tc.tile_pool(name="psum", bufs=8, space="PSUM") as psum,
            tc.tile_pool(name="dram", bufs=1, space="DRAM") as dram,
        ):
            wt = cpool.tile([P, CB * 9, CB, P], mybir.dt.float8e4)
            nc.sync.dma_start(wt[:], wb_dram[:])

            # Force the Sign ACT LUT load at kernel start (otherwise walrus
            # schedules it right before the first real sign, gating the PE).
            dummy_sg = cpool.tile([P, 1], mybir.dt.float8e4)
            nc.scalar.activation(dummy_sg[:], nc.const_aps.tensor(0.0, (P, 1)),
                                 AF.Sign)


            # per-image padded-sign tiles [128, cib, 30, 30] (fp8 for DoubleRow)
            xts = []
            sts = []
            pp = None
            for n in range(N_PER_CORE):
                st = spool.tile([P, CB, HP, WP], mybir.dt.float8e4,
                                name=f"spad{n}", tag=f"spad{n}")
                sts.append(st)

            for n in range(N_PER_CORE):
                for cb in range(CB):
                    t = n * CB + cb
                    nc.vector.memset(sts[n][:, cb], 0.0)
                    xt = xpool.tile([P, HW], mybir.dt.float32,
                                    name=f"xin{t}", tag=f"xin{t}")
                    nc.sync.dma_start(xt[:], x_dram[n, cb])
                    xts.append(xt)
                    # sign(x) into the interior of the zero-padded tile
                    nc.scalar.activation(sts[n][:, cb, 1:29, 1:29], xt[:], AF.Sign)
                if n == 0:
                    pp = cpool.tile([P, CB, 3], mybir.dt.float32)
                    nc.sync.dma_start(pp[:], pp_dram[:])

            # conv output, raw (unscaled) integer-valued sums
            z = zpool.tile([P, CB, N_PER_CORE, HW], mybir.dt.float32)
            # per-chunk row-sum partials (one column per psum tile)
            s1c = mpool.tile([P, CB, 2 * N_PER_CORE], mybir.dt.float32)
            s2c = mpool.tile([P, CB, 2 * N_PER_CORE], mybir.dt.float32)

            # Conv via fp8 DoubleRow: each matmul contracts both cin-blocks
            # (K=256) at once; several PSUM tiles accumulate per weight load
            # so each lhsT is reused and PE streams. Last groups are smaller
            # so the final stats (which gate the AllGather) finish sooner.
            GROUPS = [[(0, 0), (0, 1)],
                      [(1, 0), (1, 1)],
                      [(2, 0), (2, 1), (3, 0), (3, 1)],
                      [(4, 0), (4, 1), (5, 0), (5, 1)],
                      [(6, 0), (6, 1)],
                      [(7, 0), (7, 1)]]
            for g, units in enumerate(GROUPS):
                for cob in range(CB):
                    pss = [psum.tile([P, NFREE], mybir.dt.float32,
                                     name=f"ps_{g}_{cob}_{j}", tag="ps")
                           for j in range(len(units))]
                    for dh in range(3):
                        for dw in range(3):
                            w_ap = wt[:, cob * 9 + dh * 3 + dw, :, :]
                            first = (dh == 0 and dw == 0)
                            last = (dh == 2 and dw == 2)
                            for j, (n, half) in enumerate(units):
                                h0 = half * HALF
                                nc.tensor.matmul(
                                    pss[j][:],
                                    w_ap,
                                    sts[n][:, :, h0 + dh:h0 + dh + HALF,
                                           dw:dw + W],
                                    start=first,
                                    stop=last,
                                    perf_mode=mybir.MatmulPerfMode.DoubleRow,
                                )
                    for j, (n, half) in enumerate(units):
                        h0 = half * HALF
                        idx = n * 2 + half
                        zsl = z[:, cob, n, h0 * W:(h0 + HALF) * W]
                        nc.scalar.activation(
                            zsl, pss[j][:],
                            AF.Copy, accum_out=s1c[:, cob, idx:idx + 1])
                        sq = sqpool.tile([P, NFREE], mybir.dt.float32, tag="sq")
                        nc.vector.tensor_mul(sq[:], zsl, zsl)
                        nc.vector.tensor_reduce(
                            s2c[:, cob, idx:idx + 1], sq[:],
                            axis=mybir.AxisListType.X, op=ALU.add)

            # local stats -> [128, 4] = [s1_b0, s1_b1, s2_b0, s2_b1]
            cc_sb = mpool.tile([P, 4], mybir.dt.float32)
            nc.vector.tensor_reduce(cc_sb[:, 0:2], s1c[:],
                                    axis=mybir.AxisListType.X, op=ALU.add)
            nc.vector.tensor_reduce(cc_sb[:, 2:4], s2c[:],
                                    axis=mybir.AxisListType.X, op=ALU.add)

            # exact sync-BN: AllGather the 2KB of partial sums (lower latency
            # than AllReduce), then reduce the 8 rank contributions locally.
            cc_in = dram.tile([P, 4], mybir.dt.float32)
            ag_out = dram.tile([N_CORES, P, 4], mybir.dt.float32,
                               addr_space="Shared")
            nc.sync.dma_start(cc_in[:], cc_sb[:])
            nc.gpsimd.collective_compute(
                "AllGather", ALU.bypass,
                replica_groups=[list(range(N_CORES))],
                ins=[cc_in[:]],
                outs=[ag_out[:]],
            )
            tot8 = mpool.tile([P, N_CORES, 4], mybir.dt.float32)
            nc.sync.dma_start(tot8[:], ag_out[:].rearrange("r p c -> p r c"))
            tot = mpool.tile([P, 4], mybir.dt.float32)
            nc.vector.tensor_reduce(tot[:], tot8[:].rearrange("p r c -> p c r"),
                                    axis=mybir.AxisListType.X, op=ALU.add)

            # per-channel finalization:
            #   mu_z = S1/M ; var_z = S2/M - mu_z^2 ; var_y = scale^2*var_z
            #   A = gamma*scale/sqrt(var_y+eps) ; B = beta - A*mu_z
            inv = 1.0 / N_TOTAL_ELEMS
            mu = mpool.tile([P, CB], mybir.dt.float32)
            ez2 = mpool.tile([P, CB], mybir.dt.float32)
            m2 = mpool.tile([P, CB], mybir.dt.float32)
            varz = mpool.tile([P, CB], mybir.dt.float32)
            vary = mpool.tile([P, CB], mybir.dt.float32)
            stdv = mpool.tile([P, CB], mybir.dt.float32)
            rstd = mpool.tile([P, CB], mybir.dt.float32)
            A = mpool.tile([P, CB], mybir.dt.float32)
            t0 = mpool.tile([P, CB], mybir.dt.float32)
            B = mpool.tile([P, CB], mybir.dt.float32)

            nc.vector.tensor_scalar_mul(mu[:], tot[:, 0:2], inv)
            nc.vector.tensor_scalar_mul(ez2[:], tot[:, 2:4], inv)
            nc.vector.tensor_mul(m2[:], mu[:], mu[:])
            nc.vector.tensor_sub(varz[:], ez2[:], m2[:])
            nc.vector.tensor_mul(vary[:], varz[:], pp[:, :, 0])
            nc.vector.tensor_scalar_add(vary[:], vary[:], BN_EPS)
            nc.scalar.activation(stdv[:], vary[:], AF.Sqrt)
            nc.vector.reciprocal(rstd[:], stdv[:])
            nc.vector.tensor_mul(A[:], rstd[:], pp[:, :, 1])
            nc.vector.tensor_mul(t0[:], A[:], mu[:])
            nc.vector.tensor_sub(B[:], pp[:, :, 2], t0[:])

            # apply: out = A*z + B + x, then DMA out. The ACT scale-bias pass
            # covers two adjacent images per op (same per-channel A/B, z is
            # contiguous), pipelined with per-image DVE residual adds; the
            # out-DMA moves both images at once. GpSimd stays off SBUF ports —
            # its elementwise ops contend with DVE for the shared port pair.
            for pr in range(N_PER_CORE // 2):
                n0 = 2 * pr
                for cb in range(CB):
                    zs2 = z[:, cb, n0:n0 + 2, :]
                    nc.scalar.activation(zs2, zs2, AF.Identity,
                                         scale=A[:, cb:cb + 1],
                                         bias=B[:, cb:cb + 1])
                    for n in (n0, n0 + 1):
                        nc.vector.tensor_add(z[:, cb, n, :], z[:, cb, n, :],
                                             xts[n * CB + cb][:])
                    nc.sync.dma_start(
                        out_dram[n0:n0 + 2, cb].rearrange("n p s -> p n s"),
                        zs2)

    nc.compile()
    return nc


def _prep_shared(w, scale, gamma, beta):
    w = np.asarray(w, dtype=np.float32)
    scale = np.asarray(scale, dtype=np.float32).reshape(C)
    gamma = np.asarray(gamma, dtype=np.float32).reshape(C)
    beta = np.asarray(beta, dtype=np.float32).reshape(C)

    # DoubleRow lhsT[k, idx=(cob,dh,dw), r, m] = sign(w)[cob*128+m, r*128+k, dh, dw]
    # stored [k][idx][r][m] (contiguous per partition k) as fp8 e4m3.
    wsign = np.sign(w).astype(ml_dtypes.float8_e4m3)
    arr = wsign.reshape(CB, P, CB, P, 3, 3).transpose(3, 0, 4, 5, 2, 1)
    wb = np.ascontiguousarray(arr.reshape(P, CB * 9, CB, P))

    pp = np.empty((P, CB, 3), dtype=np.float32)
    for cb in range(CB):
        ch = slice(cb * P, (cb + 1) * P)
        pp[:, cb, 0] = scale[ch] * scale[ch]
        pp[:, cb, 1] = gamma[ch] * scale[ch]
        pp[:, cb, 2] = beta[ch]
    return wb, pp


def kernel(x, w, scale, gamma, beta):
    global _CACHED
    if _CACHED is None:
        _CACHED = _build_nc()
    nc = _CACHED

    x = np.asarray(x, dtype=np.float32)
    wb, pp = _prep_shared(w, scale, gamma, beta)

    in_maps = []
    for i in range(N_CORES):
        xs = x[i * N_PER_CORE:(i + 1) * N_PER_CORE]
        xs = np.ascontiguousarray(xs.reshape(N_PER_CORE, CB, P, HW))
        in_maps.append({"x": xs, "wb": wb, "pp": pp})

    trace = bool(int(os.environ.get("KERNEL_TRACE", "0")))
    kw = {}
    tdir = os.environ.get("KERNEL_TRACE_DIR")
    if trace and tdir:
        global _NCALL
        _NCALL = globals().get("_NCALL", 0) + 1
        tdir = os.path.join(tdir, f"call{_NCALL}")
        os.makedirs(tdir, exist_ok=True)
        kw["tmpdir"] = tdir
    res = run_bass_kernel_spmd(nc, in_maps, core_ids=list(range(N_CORES)),
                               trace=trace, **kw)
    if trace:
        globals()["LAST_EXEC_NS"] = res.exec_time_ns
        globals()["LAST_RESULTS"] = res

    out = np.empty((64, C, H, W), dtype=np.float32)
    for i in range(N_CORES):
        o = res.results[i]["out"].reshape(N_PER_CORE, C, H, W)
        out[i * N_PER_CORE:(i + 1) * N_PER_CORE] = o
    return out



# revision 37
# speedup vs baseline: 1.6048x; 1.0009x over previous
"""Trainium2 Bass kernel for a binarized BasicBlock (BinConv3x3 + scale + sync-BN + residual).

Reference computation (NCHW, N=64, C=256, H=W=28):
    out = BN_train(scale * conv3x3(sign(x), sign(w))) + x

Strategy: data-parallel over batch across 8 NeuronCores (8 images/core).
  - host: binarize weights to fp8 e4m3 DoubleRow lhsT tiles, fold gamma/scale/beta
  - device per core:
      sign(x) -> zero-padded fp8 tiles [128cin, 2cib, 30, 30] per image (ScalarE)
      conv3x3 = 9 shifted fp8 DoubleRow matmuls (K=256 per matmul) accumulated
      in PSUM; 2-4 PSUM tiles share each weight load so the PE streams gapless
      PSUM evacuation on ScalarE with row-accumulate -> per-channel sum(z);
      sum(z^2) via VectorE square+reduce
      2KB AllGather of the partial sums across the 8 cores + local reduce
      (exact sync-BN; sums of +-1 dot products are exact integers in fp32)
      per-channel A,B finalization; apply out = A*z + B + x on ScalarE/VectorE
"""

import os
import sys

sys.path.insert(0, "/opt/trn_rl_repo")

import numpy as np
import ml_dtypes

import concourse.mybir as mybir
import concourse.tile as tile
from concourse import bacc
from concourse.bass_utils import run_bass_kernel_spmd

AF = mybir.ActivationFunctionType
ALU = mybir.AluOpType

N_CORES = 8
N_PER_CORE = 8          # images per core
C = 256                 # channels
CB = 2                  # channel blocks of 128
P = 128                 # partitions
H = W = 28
HW = H * W              # 784
HP = WP = 30            # padded spatial
HALF = 14               # output rows per matmul group
NFREE = HALF * W        # 392 free elems per matmul
BN_EPS = 1e-5
N_TOTAL_ELEMS = 64 * HW  # BN normalizer: N*H*W over the full batch

_CACHED = None


def _build_nc():
    nc = bacc.Bacc("TRN2", target_bir_lowering=False, debug=False,
                   num_devices=N_CORES)

    x_dram = nc.dram_tensor("x", [N_PER_CORE, CB, P, HW], mybir.dt.float32,
                            kind="ExternalInput")
    wb_dram = nc.dram_tensor("wb", [P, CB * 9, CB, P], mybir.dt.float8e4,
                             kind="ExternalInput")
    pp_dram = nc.dram_tensor("pp", [P, CB, 3], mybir.dt.float32,
                             kind="ExternalInput")
    out_dram = nc.dram_tensor("out", [N_PER_CORE, CB, P, HW], mybir.dt.float32,
                              kind="ExternalOutput")

    with tile.TileContext(nc) as tc:
        with (
            tc.tile_pool(name="const", bufs=1) as cpool,
            tc.tile_pool(name="xin", bufs=1) as xpool,
            tc.tile_pool(name="spad", bufs=1) as spool,
            tc.tile_pool(name="z", bufs=1) as zpool,
            tc.tile_pool(name="sq", bufs=2) as sqpool,
            tc.tile_pool(name="small", bufs=1) as mpool,
            tc.tile_pool(name="psum", bufs=8, space="PSUM") as psum,
            tc.tile_pool(name="dram", bufs=1, space="DRAM") as dram,
        ):
            wt = cpool.tile([P, CB * 9, CB, P], mybir.dt.float8e4)
            nc.sync.dma_start(wt[:], wb_dram[:])

            # Force the Sign ACT LUT load at kernel start (otherwise walrus
            # schedules it right before the first real sign, gating the PE).
            dummy_sg = cpool.tile([P, 1], mybir.dt.float8e4)
            nc.scalar.activation(dummy_sg[:], nc.const_aps.tensor(0.0, (P, 1)),
                                 AF.Sign)


            # per-image padded-sign tiles [128, cib, 30, 30] (fp8 for DoubleRow)
            xts = []
            sts = []
            pp = None
            for n in range(N_PER_CORE):
                st = spool.tile([P, CB, HP, WP], mybir.dt.float8e4,
                                name=f"spad{n}", tag=f"spad{n}")
                sts.append(st)

            for n in range(N_PER_CORE):
                for cb in range(CB):
                    t = n * CB + cb
                    nc.vector.memset(sts[n][:, cb], 0.0)
                    xt = xpool.tile([P, HW], mybir.dt.float32,
                                    name=f"xin{t}", tag=f"xin{t}")
                    nc.sync.dma_start(xt[:], x_dram[n, cb])
                    xts.append(xt)
                    # sign(x) into the interior of the zero-padded tile
                    nc.scalar.activation(sts[n][:, cb, 1:29, 1:29], xt[:], AF.Sign)
                if n == 0:
                    pp = cpool.tile([P, CB, 3], mybir.dt.float32)
                    nc.sync.dma_start(pp[:], pp_dram[:])

            # conv output, raw (unscaled) integer-valued sums
            z = zpool.tile([P, CB, N_PER_CORE, HW], mybir.dt.float32)
            # per-chunk row-sum partials (one column per psum tile)
            s1c = mpool.tile([P, CB, 2 * N_PER_CORE], mybir.dt.float32)
            s2c = mpool.tile([P, CB, 2 * N_PER_CORE], mybir.dt.float32)

            # Conv via fp8 DoubleRow: each matmul contracts both cin-blocks
            # (K=256) at once; several PSUM tiles accumulate per weight load
            # so each lhsT is reused and PE streams. Last groups are smaller
            # so the final stats (which gate the AllGather) finish sooner.
            GROUPS = [[(0, 0), (0, 1)],
                      [(1, 0), (1, 1)],
                      [(2, 0), (2, 1), (3, 0), (3, 1)],
                      [(4, 0), (4, 1), (5, 0), (5, 1)],
                      [(6, 0), (6, 1)],
                      [(7, 0), (7, 1)]]
            for g, units in enumerate(GROUPS):
                for cob in range(CB):
                    pss = [psum.tile([P, NFREE], mybir.dt.float32,
                                     name=f"ps_{g}_{cob}_{j}", tag="ps")
                           for j in range(len(units))]
                    for dh in range(3):
                        for dw in range(3):
                            w_ap = wt[:, cob * 9 + dh * 3 + dw, :, :]
                            first = (dh == 0 and dw == 0)
                            last = (dh == 2 and dw == 2)
                            for j, (n, half) in enumerate(units):
                                h0 = half * HALF
                                nc.tensor.matmul(
                                    pss[j][:],
                                    w_ap,
                                    sts[n][:, :, h0 + dh:h0 + dh + HALF,
                                           dw:dw + W],
                                    start=first,
                                    stop=last,
                                    perf_mode=mybir.MatmulPerfMode.DoubleRow,
                                )
                    for j, (n, half) in enumerate(units):
                        h0 = half * HALF
                        idx = n * 2 + half
                        zsl = z[:, cob, n, h0 * W:(h0 + HALF) * W]
                        if g >= 4 and j == 1:
                            # last images: run the odd unit's evac+stats fully
                            # on DVE, in parallel with ACT handling the even
                            # unit — shortens the chain that gates the
                            # AllGather doorbell
                            nc.vector.tensor_copy(zsl, pss[j][:])
                            nc.vector.tensor_reduce(
                                s1c[:, cob, idx:idx + 1], zsl,
                                axis=mybir.AxisListType.X, op=ALU.add)
                        else:
                            nc.scalar.activation(
                                zsl, pss[j][:],
                                AF.Copy, accum_out=s1c[:, cob, idx:idx + 1])
                        if g >= 4 and j == 0:
                            sq = sqpool.tile([P, NFREE], mybir.dt.float32,
                                             tag="sqa")
                            nc.scalar.activation(
                                sq[:], pss[j][:], AF.Square,
                                accum_out=s2c[:, cob, idx:idx + 1])
                        else:
                            sq = sqpool.tile([P, NFREE], mybir.dt.float32,
                                             tag="sq")
                            nc.vector.tensor_mul(sq[:], zsl, zsl)
                            nc.vector.tensor_reduce(
                                s2c[:, cob, idx:idx + 1], sq[:],
                                axis=mybir.AxisListType.X, op=ALU.add)

            # local stats -> [128, 4] = [s1_b0, s1_b1, s2_b0, s2_b1]
            cc_sb = mpool.tile([P, 4], mybir.dt.float32)
            nc.vector.tensor_reduce(cc_sb[:, 0:2], s1c[:],
                                    axis=mybir.AxisListType.X, op=ALU.add)
            nc.vector.tensor_reduce(cc_sb[:, 2:4], s2c[:],
                                    axis=mybir.AxisListType.X, op=ALU.add)

            # exact sync-BN: AllGather the 2KB of partial sums (lower latency
            # than AllReduce), then reduce the 8 rank contributions locally.
            cc_in = dram.tile([P, 4], mybir.dt.float32)
            ag_out = dram.tile([N_CORES, P, 4], mybir.dt.float32,
                               addr_space="Shared")
            nc.sync.dma_start(cc_in[:], cc_sb[:])
            nc.gpsimd.collective_compute(
                "AllGather", ALU.bypass,
                replica_groups=[list(range(N_CORES))],
                ins=[cc_in[:]],
                outs=[ag_out[:]],
            )
            tot8 = mpool.tile([P, N_CORES, 4], mybir.dt.float32)
            nc.sync.dma_start(tot8[:], ag_out[:].rearrange("r p c -> p r c"))
            tot = mpool.tile([P, 4], mybir.dt.float32)
            nc.vector.tensor_reduce(tot[:], tot8[:].rearrange("p r c -> p c r"),
                                    axis=mybir.AxisListType.X, op=ALU.add)

            # per-channel finalization:
            #   mu_z = S1/M ; var_z = S2/M - mu_z^2 ; var_y = scale^2*var_z
            #   A = gamma*scale/sqrt(var_y+eps) ; B = beta - A*mu_z
            inv = 1.0 / N_TOTAL_ELEMS
            mm4 = mpool.tile([P, 4], mybir.dt.float32)
            m2 = mpool.tile([P, CB], mybir.dt.float32)
            varz = mpool.tile([P, CB], mybir.dt.float32)
            vary = mpool.tile([P, CB], mybir.dt.float32)
            stdv = mpool.tile([P, CB], mybir.dt.float32)
            rstd = mpool.tile([P, CB], mybir.dt.float32)
            A = mpool.tile([P, CB], mybir.dt.float32)
            t0 = mpool.tile([P, CB], mybir.dt.float32)
            B = mpool.tile([P, CB], mybir.dt.float32)

            nc.vector.tensor_scalar_mul(mm4[:], tot[:], inv)
            mu = mm4[:, 0:2]
            ez2 = mm4[:, 2:4]
            nc.vector.tensor_mul(m2[:], mu, mu)
            nc.vector.tensor_sub(varz[:], ez2, m2[:])
            nc.vector.tensor_mul(vary[:], varz[:], pp[:, :, 0])
            nc.vector.tensor_scalar_add(vary[:], vary[:], BN_EPS)
            nc.scalar.activation(stdv[:], vary[:], AF.Sqrt)
            nc.vector.reciprocal(rstd[:], stdv[:])
            nc.vector.tensor_mul(A[:], rstd[:], pp[:, :, 1])
            nc.vector.tensor_mul(t0[:], A[:], mu)
            nc.vector.tensor_sub(B[:], pp[:, :, 2], t0[:])

            # apply: out = A*z + B + x, then DMA out. The ACT scale-bias pass
            # covers two adjacent images per op (same per-channel A/B, z is
            # contiguous), pipelined with per-image DVE residual adds; the
            # out-DMA moves both images at once. GpSimd stays off SBUF ports —
            # its elementwise ops contend with DVE for the shared port pair.
            for pr in range(N_PER_CORE // 2):
                n0 = 2 * pr
                for cb in range(CB):
                    zs2 = z[:, cb, n0:n0 + 2, :]
                    nc.scalar.activation(zs2, zs2, AF.Identity,
                                         scale=A[:, cb:cb + 1],
                                         bias=B[:, cb:cb + 1])
                    for n in (n0, n0 + 1):
                        nc.vector.tensor_add(z[:, cb, n, :], z[:, cb, n, :],
                                             xts[n * CB + cb][:])
                    nc.sync.dma_start(
                        out_dram[n0:n0 + 2, cb].rearrange("n p s -> p n s"),
                        zs2)

    nc.compile()
    return nc


def _prep_shared(w, scale, gamma, beta):
    w = np.asarray(w, dtype=np.float32)
    scale = np.asarray(scale, dtype=np.float32).reshape(C)
    gamma = np.asarray(gamma, dtype=np.float32).reshape(C)
    beta = np.asarray(beta, dtype=np.float32).reshape(C)

    # DoubleRow lhsT[k, idx=(cob,dh,dw), r, m] = sign(w)[cob*128+m, r*128+k, dh, dw]
    # stored [k][idx][r][m] (contiguous per partition k) as fp8 e4m3.
    wsign = np.sign(w).astype(ml_dtypes.float8_e4m3)
    arr = wsign.reshape(CB, P, CB, P, 3, 3).transpose(3, 0, 4, 5, 2, 1)
    wb = np.ascontiguousarray(arr.reshape(P, CB * 9, CB, P))

    pp = np.empty((P, CB, 3), dtype=np.float32)
    for cb in range(CB):
        ch = slice(cb * P, (cb + 1) * P)
        pp[:, cb, 0] = scale[ch] * scale[ch]
        pp[:, cb, 1] = gamma[ch] * scale[ch]
        pp[:, cb, 2] = beta[ch]
    return wb, pp


def kernel(x, w, scale, gamma, beta):
    global _CACHED
    if _CACHED is None:
        _CACHED = _build_nc()
    nc = _CACHED

    x = np.asarray(x, dtype=np.float32)
    wb, pp = _prep_shared(w, scale, gamma, beta)

    in_maps = []
    for i in range(N_CORES):
        xs = x[i * N_PER_CORE:(i + 1) * N_PER_CORE]
        xs = np.ascontiguousarray(xs.reshape(N_PER_CORE, CB, P, HW))
        in_maps.append({"x": xs, "wb": wb, "pp": pp})

    trace = bool(int(os.environ.get("KERNEL_TRACE", "0")))
    kw = {}
    tdir = os.environ.get("KERNEL_TRACE_DIR")
    if trace and tdir:
        global _NCALL
        _NCALL = globals().get("_NCALL", 0) + 1
        tdir = os.path.join(tdir, f"call{_NCALL}")
        os.makedirs(tdir, exist_ok=True)
        kw["tmpdir"] = tdir
    res = run_bass_kernel_spmd(nc, in_maps, core_ids=list(range(N_CORES)),
                               trace=trace, **kw)
    if trace:
        globals()["LAST_EXEC_NS"] = res.exec_time_ns
        globals()["LAST_RESULTS"] = res

    out = np.empty((64, C, H, W), dtype=np.float32)
    for i in range(N_CORES):
        o = res.results[i]["out"].reshape(N_PER_CORE, C, H, W)
        out[i * N_PER_CORE:(i + 1) * N_PER_CORE] = o
    return out

